# revision 5
# baseline (speedup 1.0000x reference)
"""Cross-attention fusion block on 8 trn2 NeuronCores — v2.

Sharding: data-parallel over the query sequence (S=4096 -> 512 rows/core).
K/V projections are computed redundantly on every core (cheap vs attention).

v2 redesign vs the 213.8us baseline (profiled: ACT exp 133us and PE matmul
141us each ~65% busy, poorly overlapped):
  - AV matmul flipped to query-major output (lhsT = exp-scores, N=33 per
    head) in bf16: AV drops 131k -> 34k PE cycles.  Scores stay f32r.
  - Queries processed in two 256-column halves; the whole post-attention
    tail (normalize, transpose, out-proj, LN1, FFN, LN2, store) for half 0
    is interleaved into half 1's attention stream so only half 1's tail is
    exposed at the end.
  - ACT runs (almost) nothing but the 128 1024-element exp instructions;
    LN rstd uses exp(-0.5*ln(var+eps)) so a single activation table serves
    the whole kernel (no 1283ns table swaps).
  - bv is folded into bo on the host (attn@Wo with attn0+bv == attn0@Wo +
    (bv@Wo+bo)); bk dropped (softmax-invariant).
  - kT/v/q projections interleaved into half 0's key loop behind a
    one-block lookahead so the exp stream starts ~3us in.
"""

import sys

for _p in ("/opt/trn_rl_repo", "/opt/pypackages"):
    if _p not in sys.path:
        sys.path.append(_p)

import numpy as np

import concourse.bass as bass
import concourse.bacc as bacc
import concourse.tile as tile
from concourse import mybir
from concourse.bass_utils import run_bass_kernel_spmd

F32 = mybir.dt.float32
F32R = mybir.dt.float32r
BF16 = mybir.dt.bfloat16
AFT = mybir.ActivationFunctionType
ALU = mybir.AluOpType

P = 128           # SBUF partitions
C = 256           # channels
S = 4096          # sequence (64*64)
NCORES = 8
SH = S // NCORES  # 512 query rows per core
QH = SH // 2      # 256-query half
NH = 8            # heads
HD = 32           # head dim
HD1 = HD + 1      # V block plus the ones column for the colsum
F = 4 * C         # FFN hidden = 1024
NKC = C // P      # 2 channel chunks
NFC = F // P      # 8 ffn chunks
NSC = S // P      # 32 key chunks
EPS = 1e-5
INV_SQRT_HD = 1.0 / float(np.sqrt(HD))
INV_C = 1.0 / C
# Schraudolph-style one-op exp in bf16 bit domain:
#   bf16_bits = trunc(A*score + B);  max rel err ~3.3%
EXPA = float(128.0 * np.log2(np.e) / np.sqrt(HD))
EXPB = float(128.0 * (127.0 - 0.0435) + 0.5)
# exp engine dispatch pattern (A=ACT exact, D=DVE fast, P=Pool fast)
def _exp_engine(qb2, ck, pg):
    """One ACT + one DVE exp per key chunk (the st ring serializes
    ST(ck+1,pg) behind exp(ck,pg), so a chunk's two exps must run on
    different engines).  Exceptions: light DVE while it runs half-0's tail
    (early half 1) and pure-ACT at the end so DVE drains before the
    exposed tail."""
    if qb2 == 1 and ck >= 28:
        return 'A'
    if qb2 == 1 and ck < 10:
        return 'D' if (pg == 1 and ck % 2 == 1) else 'A'
    return 'D' if pg == 1 else 'A'


def build_bass():
    nc = bacc.Bacc()

    xT = nc.declare_dram_parameter("xT", [C, SH], F32R, isOutput=False)
    y = nc.declare_dram_parameter("y", [C, S], F32R, isOutput=False)
    w4 = nc.declare_dram_parameter("w4", [4, C, C], F32R, isOutput=False)
    w1 = nc.declare_dram_parameter("w1", [C, F], F32R, isOutput=False)
    w2 = nc.declare_dram_parameter("w2", [F, C], F32R, isOutput=False)
    ones32 = nc.declare_dram_parameter("ones32", [P, 1], F32R, isOutput=False)
    invc32 = nc.declare_dram_parameter("invc32", [P, 1], F32R, isOutput=False)
    brow = nc.declare_dram_parameter("brow", [1, 6, C], F32R,
                                    isOutput=False)
    onesrow = nc.declare_dram_parameter("onesrow", [1, SH], F32R,
                                        isOutput=False)
    eye = nc.declare_dram_parameter("eye", [P, P], F32R, isOutput=False)
    bpack = nc.declare_dram_parameter("bpack", [11, C], F32, isOutput=False)
    out = nc.declare_dram_parameter("out", [C, SH], F32, isOutput=True)

    with tile.TileContext(nc) as tc:
        _emit(tc, xT, y, w4, w1, w2, ones32, invc32, brow, onesrow, eye, bpack, out)
    if not nc.is_finalized():
        nc.finalize()
    return nc


def _emit(tc, xT, y, w4, w1, w2, ones32, invc32, brow, onesrow, eye, bpack, out):
    nc = tc.nc

    import contextlib
    stack = contextlib.ExitStack()
    with stack:
        consts = stack.enter_context(tc.tile_pool(name="consts", bufs=1))
        big = stack.enter_context(tc.tile_pool(name="big", bufs=1))
        # SBUF rings for per-half tail tensors
        ring = stack.enter_context(tc.tile_pool(name="ring", bufs=2))
        atp = stack.enter_context(tc.tile_pool(name="atp", bufs=6))
        rows = stack.enter_context(tc.tile_pool(name="rows", bufs=2))
        # PSUM
        stp = stack.enter_context(
            tc.tile_pool(name="stp", bufs=2, space="PSUM"))
        avp = stack.enter_context(
            tc.tile_pool(name="avp", bufs=2, space="PSUM"))
        tailp = stack.enter_context(
            tc.tile_pool(name="tailp", bufs=1, space="PSUM"))

        # ------------- constants / weights into SBUF -------------
        # ramp-critical order: y block0, Wk, Wq, xT gate the first exp
        y_sb = big.tile([P, NKC, S], F32R)
        y_r = y.rearrange("(kc p) s -> p kc s", p=P)
        w4_sb = consts.tile([P, 4, NKC, C], F32R)
        w4_r = w4.rearrange("w (kc p) m -> p w kc m", p=P)
        wq_sb, wk_sb, wv_sb, wo_sb = (w4_sb[:, i] for i in range(4))

        ones_rep = consts.tile([1, P], F32R)    # K=1 row-replication lhsT
        nc.sync.dma_start(out=ones_rep,
                          in_=ones32.rearrange("p one -> one p"))
        xT_sb = big.tile([P, NKC, SH], F32R)
        nc.sync.dma_start(out=xT_sb, in_=xT.rearrange("(kc p) s -> p kc s", p=P))
        nc.sync.dma_start(out=w4_sb[:, 0], in_=w4_r[:, 0])    # Wq
        for kc in range(NKC):
            nc.sync.dma_start(out=y_sb[:, kc, 0:512], in_=y_r[:, kc, 0:512])
        nc.sync.dma_start(out=w4_sb[:, 1], in_=w4_r[:, 1])    # Wk

        bp_sb = consts.tile([P, 11, NKC], F32)
        nc.sync.dma_start(
            out=bp_sb, in_=bpack.rearrange("n (kc p) -> p n kc", p=P))
        bq_sb, bo_sb, bf2_sb = (bp_sb[:, i] for i in range(3))
        g1_sb, b1_sb, g2_sb, b2_sb = (bp_sb[:, i] for i in range(3, 7))
        # bf1 rows 7..10: bf1[mf*128+p] = bpack[7 + mf//2, (mf%2)*128 + p]

        # wv next (gates the first AV), then the rest of y
        nc.sync.dma_start(out=w4_sb[:, 2], in_=w4_r[:, 2])    # Wv
        nc.sync.dma_start(out=w4_sb[:, 3], in_=w4_r[:, 3])    # Wo
        for b in range(1, 8):
            sl = slice(b * 512, (b + 1) * 512)
            for kc in range(NKC):
                nc.sync.dma_start(out=y_sb[:, kc, sl], in_=y_r[:, kc, sl])

        ones1r = consts.tile([P, 1], F32R)      # LN-stats lhsT (1/C each)
        nc.sync.dma_start(out=ones1r, in_=invc32[:])
        brow_sb = consts.tile([1, 6, C], F32R)  # bo', bf2, g1, -b1, g2, -b2
        nc.sync.dma_start(out=brow_sb, in_=brow[:])
        onesrow_sb = consts.tile([1, SH], F32R)
        nc.sync.dma_start(out=onesrow_sb, in_=onesrow[:])
        eye_sb = consts.tile([P, P], F32R)      # transpose identity
        nc.sync.dma_start(out=eye_sb, in_=eye[:])
        eps_sb = consts.tile([1, 1], F32)
        nc.vector.memset(eps_sb, EPS)
        warm = consts.tile([1, 1], F32)
        # Warm the exp activation table off the critical path.
        nc.scalar.activation(out=warm, in_=eps_sb, func=AFT.Exp)

        # persistent activations
        qT_sb = big.tile([P, NKC, SH], F32R)    # q^T  (with bq)
        kT_sb = big.tile([P, NKC, S], F32R)     # k^T  (no bk; softmax-invariant)
        v2_sb = big.tile([P, NSC, NH, HD1], BF16)  # per head [V_h | 1]
        attnT_sb = big.tile([P, NKC, SH], F32R)  # attention output, ch-major

        # ones column of v2
        nc.vector.memset(v2_sb[:, :, :, HD:HD1], 1.0)

        # ---------------- prologue: qT; kT/v block 0 ----------------
        def proj_q():
            ps = stp.tile([P, 4, QH], F32, tag="st", name="ps_q")
            for mc in range(NKC):
                for kc in range(NKC):
                    nc.tensor.matmul(
                        ps[:, 2 * mc:2 * mc + 2, :],
                        wq_sb[:, kc, mc * P:(mc + 1) * P],
                        xT_sb[:, kc, :],
                        start=(kc == 0), stop=(kc == NKC - 1))
            for mc in range(NKC):
                nc.vector.tensor_scalar_add(
                    out=qT_sb[:, mc, :], in0=ps[:, 2 * mc:2 * mc + 2, :],
                    scalar1=bq_sb[:, mc:mc + 1])

        def proj_k_block(b):
            """k^T columns [512b, 512b+512) via the tailp bank."""
            sl = slice(b * 512, (b + 1) * 512)
            for mc in range(NKC):
                ps = tailp.tile([P, NKC, QH], F32, tag="tp",
                                name=f"tp_k{b}_{mc}")
                for kc in range(NKC):
                    nc.tensor.matmul(
                        ps[:, :, :],
                        wk_sb[:, kc, mc * P:(mc + 1) * P],
                        y_sb[:, kc, sl],
                        start=(kc == 0), stop=(kc == NKC - 1))
                nc.scalar.copy(kT_sb[:, mc, sl], ps[:, :, :])

        def proj_v_block(ck0):
            """v rows [128*ck0, 128*(ck0+2)) -> v2 (2 key chunks)."""
            ps = tailp.tile([P, NKC, QH], F32, tag="tp", name=f"tp_v{ck0}")
            for j in range(2):
                ck = ck0 + j
                for kc in range(NKC):
                    nc.tensor.matmul(
                        ps[:, j, :],
                        y_sb[:, kc, ck * P:(ck + 1) * P],
                        wv_sb[:, kc, :],
                        start=(kc == 0 and j == 0),
                        stop=(kc == NKC - 1 and j == 1))
            src = ps[:, :, :].rearrange("p a (h d) -> p a h d", d=HD)
            nc.vector.tensor_copy(v2_sb[:, ck0:ck0 + 2, :, 0:HD], src)

        # start the PE p-state ramp clock as early as possible
        warm_ps = stp.tile([P, 4, QH], F32, tag="st", name="warm_ps")
        nc.tensor.matmul(warm_ps[:, 0, 0:P], ones_rep, ones_rep,
                         start=True, stop=True)
        proj_q()
        proj_k_block(0)
        # FFN weights can trickle in during attention
        w1_sb = consts.tile([P, NKC, F], F32R)
        nc.sync.dma_start(out=w1_sb, in_=w1.rearrange("(kc p) m -> p kc m", p=P))
        w2_sb = consts.tile([P, NFC, C], F32R)
        nc.sync.dma_start(out=w2_sb, in_=w2.rearrange("(kc p) m -> p kc m", p=P))

        # ---------------- attention + pipelined tail ----------------
        out_r = out.rearrange("(kc p) s -> p kc s", p=P)

        def make_tail(qb2, avq, act_ok):
            """Return list of emission closures for this half's tail."""
            qsl = slice(qb2 * QH, (qb2 + 1) * QH)
            attn_rows = ring.tile([P, 2, C], F32R, tag="attn_rows",
                                  name=f"attn_rows{qb2}")
            rec = ring.tile([P, 2, NH], F32, tag="rec", name=f"rec{qb2}")
            scr = ring.tile([P, 2, NH], F32, tag="scr", name=f"scr{qb2}")
            r_t = ring.tile([P, NKC, QH], F32R, tag="r", name=f"r{qb2}")
            z_t = ring.tile([P, NKC, QH], F32R, tag="z", name=f"z{qb2}")
            r2_t = ring.tile([P, NKC, QH], F32R, tag="r2", name=f"r2{qb2}")
            out_t = ring.tile([P, NKC, QH], F32, tag="out", name=f"out{qb2}")
            h1_t = ring.tile([P, NFC, QH], F32R, tag="h1", name=f"h1{qb2}")

            def norm_rec():
                for qc in range(2):
                    nc.vector.reciprocal_approx_accurate(
                        out=rec[:, qc, :], in_=avq[qc][:, :, HD],
                        scratch=scr[:, qc, :])

            def norm_mul():
                for qc in range(2):
                    for h in range(NH):
                        dst = attn_rows[:, qc, h * HD:(h + 1) * HD]
                        src = avq[qc][:, h, 0:HD]
                        if act_ok and h % 2 == 0:
                            nc.scalar.activation(
                                out=dst, in_=src, func=AFT.Copy,
                                scale=rec[:, qc, h:h + 1])
                        else:
                            nc.vector.tensor_scalar_mul(
                                out=dst, in0=src,
                                scalar1=rec[:, qc, h:h + 1])

            def transp():
                tp = tailp.tile([P, NKC, QH], F32, tag="tp",
                                name=f"tp_t{qb2}")
                for kc in range(NKC):
                    for qc in range(2):
                        nc.tensor.matmul(
                            tp[:, kc, qc * P:(qc + 1) * P].bitcast(F32R),
                            attn_rows[:, qc, kc * P:(kc + 1) * P],
                            eye_sb, is_transpose=True,
                            start=(kc == 0 and qc == 0),
                            stop=(kc == 1 and qc == 1))
                for kc in range(NKC):
                    if act_ok and kc == 0:
                        nc.scalar.copy(attnT_sb[:, kc, qsl], tp[:, kc, :])
                    else:
                        nc.vector.tensor_copy(attnT_sb[:, kc, qsl],
                                              tp[:, kc, :])

            def outproj():
                ps = tailp.tile([P, NKC, QH], F32, tag="tp",
                                name=f"tp_o{qb2}")
                for mc in range(NKC):
                    for kc in range(NKC):
                        nc.tensor.matmul(
                            ps[:, mc, :],
                            wo_sb[:, kc, mc * P:(mc + 1) * P],
                            attnT_sb[:, kc, qsl],
                            start=(kc == 0 and mc == 0), stop=False)
                    # + qT residual (identity matmul) + bo' (outer product)
                    nc.tensor.matmul(ps[:, mc, :], eye_sb, qT_sb[:, mc, qsl],
                                     start=False, stop=False)
                    nc.tensor.matmul(ps[:, mc, :],
                                     brow_sb[0:1, 0, mc * P:(mc + 1) * P],
                                     onesrow_sb[0:1, 0:QH],
                                     start=False, stop=(mc == NKC - 1))
                for mc in range(NKC):
                    if act_ok and mc == 0:
                        nc.scalar.copy(r_t[:, mc, :], ps[:, mc, :])
                    else:
                        nc.vector.tensor_copy(r_t[:, mc, :], ps[:, mc, :])

            def layer_norm_steps(x3, gi, out3, nm):
                """gi: brow row of gamma (gi+1 holds -beta)."""
                ee = nc.vector  # Pool lacks these ops in codegen
                lnps = tailp.tile([P, NKC, QH], F32, tag="lnps",
                                  name=f"lnps_{nm}{qb2}")
                m_row = rows.tile([1, QH], F32R, tag="m_row",
                                  name=f"m_row{qb2}{nm}")
                m2_row = rows.tile([1, QH], F32, tag="m2_row",
                                   name=f"m2_row{qb2}{nm}")
                var_row = rows.tile([1, QH], F32, tag="var_row",
                                    name=f"var_row{qb2}{nm}")
                r0_row = rows.tile([1, QH], F32, tag="r0_row",
                                   name=f"r0_row{qb2}{nm}")
                nt_row = rows.tile([1, QH], F32, tag="nt_row",
                                   name=f"nt_row{qb2}{nm}")
                rstd_row = rows.tile([1, QH], F32R, tag="rstd_row",
                                     name=f"rstd_row{qb2}{nm}")
                rsm_row = rows.tile([1, QH], F32R, tag="rsm_row",
                                    name=f"rsm_row{qb2}{nm}")

                def stats():
                    # ones1r carries 1/C: chains emit mean and E[x^2] rows
                    for kc in range(NKC):
                        nc.tensor.matmul(lnps[0:1, 0, :], ones1r, x3[:, kc, :],
                                         start=(kc == 0), stop=False)
                    sq = rows.tile([P, NKC, QH], F32R, tag="sq",
                                   name=f"sq{qb2}{nm}")
                    ee.tensor_mul(sq, x3, x3)
                    for kc in range(NKC):
                        nc.tensor.matmul(lnps[0:1, 1, :], ones1r, sq[:, kc, :],
                                         start=False, stop=(kc == NKC - 1))

                def rws():
                    nc.vector.tensor_copy(m_row, lnps[0:1, 0, :])
                    nc.vector.tensor_mul(m2_row, m_row, m_row)
                    nc.vector.scalar_tensor_tensor(
                        out=var_row, in0=lnps[0:1, 1, :], scalar=1.0,
                        in1=m2_row, op0=ALU.mult, op1=ALU.subtract)

                def rstd():
                    # magic-constant rsqrt + one Newton step (SBUF-only, so
                    # it can run on GPSIMD for the pipelined half)
                    u32 = mybir.dt.uint32
                    ee.tensor_scalar(
                        out=r0_row.bitcast(u32), in0=var_row.bitcast(u32),
                        scalar1=1, scalar2=None,
                        op0=ALU.logical_shift_right)
                    ee.tensor_scalar(
                        out=r0_row.bitcast(u32), in0=r0_row.bitcast(u32),
                        scalar1=-1.0, scalar2=float(0x5f3759df),
                        op0=ALU.mult, op1=ALU.add)
                    # nt = 1.5 - 0.5*v*r0^2 ; rstd = r0*nt ; rsm = rstd*mean
                    ee.tensor_mul(nt_row, r0_row, r0_row)
                    ee.tensor_mul(nt_row, nt_row, var_row)
                    ee.tensor_scalar(
                        out=nt_row, in0=nt_row, scalar1=-0.5, scalar2=1.5,
                        op0=ALU.mult, op1=ALU.add)
                    ee.tensor_mul(rstd_row, r0_row, nt_row)
                    ee.tensor_mul(rsm_row, rstd_row, m_row)

                def affine():
                    # A = g (x) rstd ; B = g (x) rstd*mu + (-b) (x) ones
                    # out = x*A - B
                    for kc in range(NKC):
                        grow = brow_sb[0:1, gi, kc * P:(kc + 1) * P]
                        nc.tensor.matmul(lnps[:, kc, :], grow, rstd_row,
                                         start=(kc == 0), stop=(kc == 1))
                    bt = tailp.tile([P, NKC, QH], F32, tag="tp",
                                    name=f"tp_b{qb2}{nm}")
                    for kc in range(NKC):
                        grow = brow_sb[0:1, gi, kc * P:(kc + 1) * P]
                        nbrow = brow_sb[0:1, gi + 1, kc * P:(kc + 1) * P]
                        nc.tensor.matmul(bt[:, kc, :], grow, rsm_row,
                                         start=(kc == 0), stop=False)
                        nc.tensor.matmul(bt[:, kc, :], nbrow,
                                         onesrow_sb[0:1, 0:QH],
                                         start=False, stop=(kc == 1))
                    t = rows.tile([P, NKC, QH], F32R, tag="t",
                                  name=f"t{qb2}{nm}")
                    nc.vector.tensor_mul(t, x3, lnps)
                    nc.vector.tensor_sub(out3, t, bt)

                return [stats, rws, rstd, affine]

            ln1 = layer_norm_steps(r_t, 2, z_t, "a")

            def ffn1_relu(dst, src, bf1, on_act):
                if on_act:
                    nc.scalar.activation(out=dst, in_=src, func=AFT.Relu,
                                         bias=bf1)
                else:
                    nc.vector.tensor_scalar(
                        out=dst, in0=src, scalar1=bf1, scalar2=0.0,
                        op0=ALU.add, op1=ALU.max)

            def ffn1_pair(mf0):
                """tail0 path: 2 ffn1 chains through the 1-bank tp tile."""
                ps = tailp.tile([P, NKC, QH], F32, tag="tp",
                                name=f"tp_f{qb2}_{mf0}")
                for i in range(2):
                    mf = mf0 + i
                    for kc in range(NKC):
                        nc.tensor.matmul(
                            ps[:, i, :],
                            w1_sb[:, kc, mf * P:(mf + 1) * P],
                            z_t[:, kc, :],
                            start=(kc == 0 and i == 0),
                            stop=(kc == NKC - 1 and i == 1))
                for i in range(2):
                    mf = mf0 + i
                    bf1 = bp_sb[:, 7 + mf // 2, mf % 2:mf % 2 + 1]
                    ffn1_relu(h1_t[:, mf, :], ps[:, i, :], bf1, False)

            def ffn1_wide():
                """tail1 path: stp ring is free once attention is done."""
                for half in range(2):
                    ps = stp.tile([P, 4, QH], F32, tag="st",
                                  name=f"st_f{qb2}_{half}")
                    for i in range(4):
                        mf = 4 * half + i
                        for kc in range(NKC):
                            nc.tensor.matmul(
                                ps[:, i, :],
                                w1_sb[:, kc, mf * P:(mf + 1) * P],
                                z_t[:, kc, :],
                                start=(kc == 0 and i % 2 == 0),
                                stop=(kc == NKC - 1 and i % 2 == 1))
                    for i in range(4):
                        mf = 4 * half + i
                        bf1 = bp_sb[:, 7 + mf // 2, mf % 2:mf % 2 + 1]
                        ffn1_relu(h1_t[:, mf, :], ps[:, i, :], bf1, i % 2 == 0)

            def ffn2():
                ps = tailp.tile([P, NKC, QH], F32, tag="tp",
                                name=f"tp_g{qb2}")
                for mc in range(NKC):
                    for kf in range(NFC):
                        nc.tensor.matmul(
                            ps[:, mc, :],
                            w2_sb[:, kf, mc * P:(mc + 1) * P],
                            h1_t[:, kf, :],
                            start=(kf == 0 and mc == 0), stop=False)
                    nc.tensor.matmul(ps[:, mc, :], eye_sb, z_t[:, mc, :],
                                     start=False, stop=False)
                    nc.tensor.matmul(ps[:, mc, :],
                                     brow_sb[0:1, 1, mc * P:(mc + 1) * P],
                                     onesrow_sb[0:1, 0:QH],
                                     start=False, stop=(mc == NKC - 1))
                for mc in range(NKC):
                    if act_ok and mc == 0:
                        nc.scalar.copy(r2_t[:, mc, :], ps[:, mc, :])
                    else:
                        nc.vector.tensor_copy(r2_t[:, mc, :], ps[:, mc, :])

            ln2 = layer_norm_steps(r2_t, 4, out_t, "b")

            def store():
                nc.sync.dma_start(out=out_r[:, 0, qsl], in_=out_t[:, 0, :])
                nc.sync.dma_start(out=out_r[:, 1, qsl], in_=out_t[:, 1, :])

            if act_ok:
                ffn1_steps = [ffn1_wide]
            else:
                ffn1_steps = [lambda m=m: ffn1_pair(m) for m in (0, 2, 4, 6)]
            return ([norm_rec, norm_mul, transp, outproj,
                     ln1[0], ln1[1], None, ln1[2], ln1[3]]
                    + ffn1_steps
                    + [ffn2, ln2[0], ln2[1], None, ln2[2], ln2[3], store])

        tail0 = None
        for qb2 in range(2):
            qsl = slice(qb2 * QH, (qb2 + 1) * QH)
            avq = [avp.tile([P, NH, HD1], F32, tag="av",
                            name=f"av{qb2}_{qc}") for qc in range(2)]
            def emit_st(ck, pg):
                st = stp.tile([P, 4, QH], F32, tag="st",
                              name=f"st{qb2}_{ck}_{pg}")
                for j in range(4):
                    po = HD * j
                    nc.tensor.matmul(
                        st[:, j, :],
                        kT_sb[po:po + HD, pg, ck * P:(ck + 1) * P],
                        qT_sb[po:po + HD, pg, qsl],
                        start=(j % 2 == 0), stop=(j % 2 == 1),
                        tile_position=(po, 0))
                at = atp.tile([P, 4, QH], BF16, tag="at")
                kind = _exp_engine(qb2, ck, pg)
                if kind == 'A':
                    nc.scalar.activation(out=at, in_=st, func=AFT.Exp,
                                         scale=INV_SQRT_HD)
                else:
                    eng = nc.vector if kind == 'D' else nc.gpsimd
                    eng.tensor_scalar(
                        out=at.bitcast(mybir.dt.uint16), in0=st,
                        scalar1=EXPA, scalar2=EXPB,
                        op0=ALU.mult, op1=ALU.add)
                return at

            def emit_av(ck, pg, at):
                for j in range(4):
                    h = 4 * pg + j
                    for qc in range(2):
                        nc.tensor.matmul(
                            avq[qc][:, h, :],
                            at[:, j, qc * P:(qc + 1) * P],
                            v2_sb[:, ck, h, :],
                            start=(ck == 0 and h == 0),
                            stop=(ck == NSC - 1 and h == NH - 1))

            pend = []  # deferred AV emissions: PE must never sit behind
            #            the current chunk's exps, so AVs lag one chunk

            def drain_pend(keep):
                while len(pend) > keep:
                    cki, pgi, ati = pend.pop(0)
                    emit_av(cki, pgi, ati)

            for ck in range(NSC):
                if qb2 == 0 and ck == 0:
                    # ramp: score the first chunk before projecting v
                    pend.append((0, 0, emit_st(0, 0)))
                    pend.append((0, 1, emit_st(0, 1)))
                    proj_v_block(0)
                    proj_v_block(2)
                    proj_k_block(1)
                    continue
                if qb2 == 0:
                    if ck % 4 == 0 and ck // 4 + 1 < 8:
                        proj_k_block(ck // 4 + 1)
                    if ck % 2 == 0 and ck + 4 <= NSC:
                        proj_v_block(ck + 2)
                for pg in range(2):
                    pend.append((ck, pg, emit_st(ck, pg)))
                    drain_pend(2)
                if qb2 == 1 and tail0 is not None:
                    # half-0's tail: two steps per key chunk, early
                    for i in (2 * ck, 2 * ck + 1):
                        if i < len(tail0) and tail0[i] is not None:
                            tail0[i]()
            drain_pend(0)
            if qb2 == 0:
                tail0 = make_tail(0, avq, act_ok=False)
            else:
                for step in tail0[NSC:]:
                    if step is not None:
                        step()
                tail1 = make_tail(1, avq, act_ok=True)
                for step in tail1:
                    if step is not None:
                        step()


_NC_CACHE = None


def _get_nc():
    global _NC_CACHE
    if _NC_CACHE is None:
        _NC_CACHE = build_bass()
    return _NC_CACHE


def make_in_maps(lidar_features, image_features, Wq, bq, Wk, bk, Wv, bv,
                 Wo, bo, g1, b1, W1, bf1, W2, bf2, g2, b2):
    xT_full = np.ascontiguousarray(
        np.asarray(lidar_features, np.float32).reshape(C, S))
    y_full = np.ascontiguousarray(
        np.asarray(image_features, np.float32).reshape(C, S))
    w4 = np.ascontiguousarray(np.stack([
        np.asarray(Wq, np.float32), np.asarray(Wk, np.float32),
        np.asarray(Wv, np.float32), np.asarray(Wo, np.float32)]))
    # fold bv through the out-projection: (attn0 + bv) @ Wo + bo
    bo_eff = (np.asarray(bv, np.float32) @ np.asarray(Wo, np.float32)
              + np.asarray(bo, np.float32))
    bpack = np.ascontiguousarray(np.concatenate([
        np.asarray(bq, np.float32)[None], bo_eff[None],
        np.asarray(bf2, np.float32)[None],
        np.asarray(g1, np.float32)[None], np.asarray(b1, np.float32)[None],
        np.asarray(g2, np.float32)[None], np.asarray(b2, np.float32)[None],
        np.asarray(bf1, np.float32).reshape(4, C)]))
    brow = np.ascontiguousarray(np.stack([
        bo_eff, np.asarray(bf2, np.float32),
        np.asarray(g1, np.float32), -np.asarray(b1, np.float32),
        np.asarray(g2, np.float32), -np.asarray(b2, np.float32)]))
    common = {
        "y": y_full,
        "w4": w4,
        "w1": np.ascontiguousarray(np.asarray(W1, np.float32)),
        "w2": np.ascontiguousarray(np.asarray(W2, np.float32)),
        "ones32": np.ones((P, 1), np.float32),
        "invc32": np.full((P, 1), 1.0 / C, np.float32),
        "brow": brow[None],
        "onesrow": np.ones((1, SH), np.float32),
        "eye": np.eye(P, dtype=np.float32),
        "bpack": bpack,
    }
    in_maps = []
    for c in range(NCORES):
        m = dict(common)
        m["xT"] = np.ascontiguousarray(xT_full[:, c * SH:(c + 1) * SH])
        in_maps.append(m)
    return in_maps


def kernel(lidar_features, image_features, Wq, bq, Wk, bk, Wv, bv, Wo, bo,
           g1, b1, W1, bf1, W2, bf2, g2, b2, num_heads, **run_kwargs):
    assert int(num_heads) == NH
    nc = _get_nc()
    in_maps = make_in_maps(lidar_features, image_features, Wq, bq, Wk, bk,
                           Wv, bv, Wo, bo, g1, b1, W1, bf1, W2, bf2, g2, b2)
    res = run_bass_kernel_spmd(nc, in_maps, core_ids=list(range(NCORES)),
                               **run_kwargs)
    full = np.concatenate([res.results[c]["out"] for c in range(NCORES)],
                          axis=1)
    kernel.last_results = res
    return full.reshape(1, C, 64, 64).astype(np.float32)


kernel.last_results = None
